# revision 27
# baseline (speedup 1.0000x reference)
"""Distributed causal self-attention kernel for one TRN2 chip (8 NeuronCores).

Problem: y = CausalSelfAttention(x) with B=2, T=2048, C=1024, 16 heads x 64.

Sharding (per core c = b*4 + hg;  b = batch, hg = head-group of 4 heads):
  - Q/K/V projections: column-sharded per head group (each core computes its
    4 heads' Q,K,V from the full x of its batch).
  - Attention: fully local (4 heads per core as 2 pairs), flash-style, scores
    kept transposed (s^T[k, q]) so no on-chip transposes are needed.
  - Row-sums for softmax ride the AV matmul as a 65th "ones" column of V.
  - y^T shards are AllGathered within each batch group of 4 cores (pair 0 in
    two T-half gathers issued mid-attention; pair 1 per query-tile so only the
    last quarter-gather is exposed in the tail).
  - o_proj: each core computes its own 256 output columns from the gathered
    y^T; even c-blocks (pair 0) accumulate first so only the odd half waits
    on the final gather.

Scheduling: the Tile scheduler pops ready work per engine by priority
(emission order).  Priorities are banded so the QK->exp spine saturates ACT,
AV/normalize trail just behind, and pair-1 projections + o_proj fill PE gaps.
PSUM is split into three pools (aug 2 banks / score supertiles 4 banks /
projection+o_proj 2 banks) so projections never serialize against attention.

All matmuls run in bf16 (fp32 accumulation in PSUM); inputs are converted to
bf16 on the host. QK^T matmuls (contraction dim 64) are packed two-per-PE
via tile_position row tiling.
"""
import sys
sys.path.insert(0, '/opt/trn_rl_repo')
import numpy as np
import ml_dtypes

B, T, C = 2, 2048, 1024
NH, HD = 16, 64
N_CORES = 8
GROUPS = [[0, 1, 2, 3], [4, 5, 6, 7]]
HPC = NH // 4            # heads per core = 4
SH = HPC * HD            # per-core projection width = 256
NCB = C // 128           # contraction blocks = 8
QT = 512                 # query tile
BF16 = ml_dtypes.bfloat16

_CACHE = {}


def _build(t_len):
    import concourse.bass as bass
    import concourse.bacc as bacc
    import concourse.tile as tile
    import concourse.mybir as mybir
    from contextlib import contextmanager
    dt = mybir.dt
    f32, bf16 = dt.float32, dt.bfloat16

    nqt = t_len // QT        # query tiles = 4
    ntc = t_len // 128       # t chunks of 128 = 16
    VW = HPC * 65            # vhat row width = 260

    nc = bacc.Bacc("TRN2", target_bir_lowering=False, debug=False,
                   num_devices=N_CORES)
    xT = nc.dram_tensor("xT", [128, NCB * t_len], bf16, kind="ExternalInput")
    wq = nc.dram_tensor("wqT", [128, NCB * SH], bf16, kind="ExternalInput")
    wk = nc.dram_tensor("wkT", [128, NCB * SH], bf16, kind="ExternalInput")
    wv = nc.dram_tensor("wvT", [128, NCB * SH], bf16, kind="ExternalInput")
    wo = nc.dram_tensor("woT", [128, NCB * SH], bf16, kind="ExternalInput")
    masks = nc.dram_tensor("masks", [128, 1024], bf16, kind="ExternalInput")
    out = nc.dram_tensor("out", [SH, t_len], bf16, kind="ExternalOutput")

    with tile.TileContext(nc) as tc:
        # Priority bands: the scheduler pops the lowest-priority READY
        # instruction per engine, so band order decides who wins contention.
        cursors = {"dma": 0, "spine": 1 << 20, "trail": 2 << 20,
                   "fill": 3 << 20, "tail": 4 << 20, "idle": 5 << 20}

        @contextmanager
        def band(name):
            saved = tc.cur_priority
            tc.cur_priority = cursors[name]
            try:
                yield
            finally:
                cursors[name] = tc.cur_priority
                tc.cur_priority = saved

        with tc.tile_pool(name="big", bufs=1) as big, \
             tc.tile_pool(name="epool", bufs=10) as epool, \
             tc.tile_pool(name="small", bufs=4) as small, \
             tc.tile_pool(name="yg0p", bufs=2) as yg0p, \
             tc.tile_pool(name="yg1p", bufs=4) as yg1p, \
             tc.tile_pool(name="stp", bufs=3) as stp, \
             tc.tile_pool(name="pa", bufs=1, space="PSUM") as pa, \
             tc.tile_pool(name="pq", bufs=2, space="PSUM") as pq, \
             tc.tile_pool(name="pp", bufs=2, space="PSUM") as pp, \
             tc.tile_pool(name="dram", bufs=1, space="DRAM") as dram:

            # ---- resident SBUF tensors ----
            xt = big.tile([128, NCB * t_len], bf16)       # x^T, c-blocked
            wq_sb = big.tile([128, NCB * SH], bf16)
            wk_sb = big.tile([128, NCB * SH], bf16)
            wv_sb = big.tile([128, NCB * SH], bf16)
            wo_sb = big.tile([128, NCB * SH], bf16)
            mask_sb = big.tile([128, 1024], bf16)
            qt_sb = big.tile([128, 2 * t_len], bf16)      # Q^T, pair-blocked
            kt_sb = big.tile([128, 2 * t_len], bf16)
            vhat_sb = big.tile([128, ntc * VW], bf16)     # [V_h | 1] per head

            # ---- input DMA, first-need order ----
            # wq/wk arrive pair-major ([pair][block][128]) so each head
            # pair's full contraction is one contiguous 256KB load; x^T in
            # eighths (all 8 c-blocks x 256 t cols, contiguous 512B runs).
            def w_dma(sb, src, half):
                s = half * 4 * SH
                nc.sync.dma_start(sb[:, s:s + 4 * SH], src[:, s:s + 4 * SH])

            def x_dma(e):
                # eighth e covers t cols [e*256, (e+1)*256) of every c-block
                v = xt.rearrange("p (k t) -> p k t", k=NCB)
                s = xT.rearrange("p (k t) -> p k t", k=NCB)
                nc.sync.dma_start(v[:, :, e * 256:(e + 1) * 256],
                                  s[:, :, e * 256:(e + 1) * 256])

            with band("dma"):
                w_dma(wq_sb, wq, 0)      # pair 0 of Wq
                x_dma(0)
                x_dma(1)
                w_dma(wk_sb, wk, 0)      # pair 0 of Wk
                nc.sync.dma_start(mask_sb[:], masks[:])
                w_dma(wv_sb, wv, 0)
                w_dma(wv_sb, wv, 1)
                # only the per-head "ones" columns of vhat need initializing;
                # the value columns are fully overwritten by the V copies.
                nc.gpsimd.memset(
                    vhat_sb.rearrange("p (c x) -> p c x", x=65)[:, :, 64:65], 1.0)
                x_dma(2)
                x_dma(3)
                w_dma(wq_sb, wq, 1)      # pair 1 of Wq
                w_dma(wk_sb, wk, 1)      # pair 1 of Wk
                x_dma(4)
                x_dma(5)
                x_dma(6)
                x_dma(7)
                w_dma(wo_sb, wo, 0)
                w_dma(wo_sb, wo, 1)

            # ---- PE warmup ----
            # The PE p-state ramps over ~3us of continuous busy time; work
            # dispatched into a cold PE runs 2-3.7x slower.  A chain of tiny
            # accumulating matmuls (never read) keeps the PE busy while the
            # input DMAs land and while the tail waits on collectives, so
            # real matmuls always dispatch fully ramped.  The chain sits in
            # the lowest-precedence band, so it only runs when nothing real
            # is ready.
            with band("dma"):
                wu = small.tile([128, 64], bf16, name="wu")
                nc.gpsimd.memset(wu[:], 0.5)

            def warmup(n_links, pool, shape, tag):
                # "idle" band: loses every contention, so links only run in
                # PE bubbles. Each link is ~98ns of engine time; size chains
                # so they exhaust before the phase's idle window ends.
                with band("idle"):
                    ps = pool.tile(shape, f32, name=tag)
                    for i in range(n_links):
                        nc.tensor.matmul(ps[0:16, 0:64],
                                         lhsT=wu[:, 0:16], rhs=wu[:],
                                         start=(i == 0), stop=(i == n_links - 1))

            warmup(90, pp, [128, 512], "pp")

            # ---- DRAM bounce buffers for the AllGathers ----
            # pair 0: two T-half gathers; pair 1: per-qi quarter gathers.
            agin0 = [dram.tile([128, 1024], bf16, name=f"agin0{th}")
                     for th in range(2)]
            agout0 = [dram.tile([512, 1024], bf16, name=f"agout0{th}")
                      for th in range(2)]
            agin1 = [dram.tile([128, QT], bf16, name=f"agin1{qi}")
                     for qi in range(nqt)]
            agout1 = [dram.tile([512, QT], bf16, name=f"agout1{qi}")
                      for qi in range(nqt)]

            # ---- projection helpers ----
            def qk_proj_tile(pair, w_sb, dst_sb, n, bnd):
                """One 512-col tile of Q^T/K^T for a head pair."""
                with band(bnd):
                    ps = pp.tile([128, 512], f32, name="pp")
                    pbase = pair * NCB * 128
                    for k in range(NCB):
                        nc.tensor.matmul(
                            ps[:],
                            lhsT=w_sb[:, pbase + k * 128: pbase + (k + 1) * 128],
                            rhs=xt[:, k * t_len + n * QT: k * t_len + n * QT + QT],
                            start=(k == 0), stop=(k == NCB - 1))
                    nc.vector.tensor_copy(
                        dst_sb[:, pair * t_len + n * QT: pair * t_len + n * QT + QT],
                        ps[:])

            def v_chunk(tch):
                """V for one 128-t chunk, all 4 heads, strided into vhat."""
                with band("trail"):
                    ps = pp.tile([128, 512], f32, name="pp")
                    for k in range(NCB):
                        nc.tensor.matmul(
                            ps[:, 0:SH],
                            lhsT=xt[:, k * t_len + tch * 128: k * t_len + (tch + 1) * 128],
                            rhs=wv_sb[:, k * SH:(k + 1) * SH],
                            start=(k == 0), stop=(k == NCB - 1))
                    vv = vhat_sb[:, tch * VW:(tch + 1) * VW].rearrange(
                        "p (h x) -> p h x", h=HPC)
                    nc.vector.tensor_copy(
                        vv[:, :, 0:64],
                        ps[:, 0:SH].rearrange("p (h x) -> p h x", h=HPC))

            # ---- attention ----
            def qk_mm(dst, pair, kb, qa, w, h01):
                nc.tensor.matmul(
                    dst,
                    lhsT=kt_sb[h01 * 64:(h01 + 1) * 64,
                               pair * t_len + kb * 128: pair * t_len + (kb + 1) * 128],
                    rhs=qt_sb[h01 * 64:(h01 + 1) * 64,
                              pair * t_len + qa: pair * t_len + qa + w],
                    start=True, stop=True,
                    tile_position=(h01 * 64, 0))

            def av_mm(aug, pair, e_slice, kb, h01, ca, w, start, stop):
                h = pair * 2 + h01
                return nc.tensor.matmul(
                    aug[0:65, h01 * 512 + ca: h01 * 512 + ca + w],
                    lhsT=vhat_sb[:, kb * VW + h * 65: kb * VW + (h + 1) * 65],
                    rhs=e_slice,
                    start=start, stop=stop,
                    skip_group_check=True)

            def attention_qi(pair, qi):
                """Emit one query tile's jobs: spine (qk+exp[+mask]) in the
                spine band, AV + normalize + y-DMA in the trail band."""
                q0 = qi * QT
                nfull = q0 // 128
                with band("trail"):
                    aug = pa.tile([128, 1024], f32, name="aug")
                # full supertiles
                for kb in range(nfull):
                    with band("spine"):
                        qk = pq.tile([128, 1024], f32, name="qk")
                        for h01 in (0, 1):
                            qk_mm(qk[:, h01 * 512:(h01 + 1) * 512], pair, kb,
                                  q0, 512, h01)
                        e = epool.tile([128, 1024], bf16, name="e")
                        nc.scalar.activation(e[:], qk[:],
                                             mybir.ActivationFunctionType.Exp,
                                             scale=1.0 / np.sqrt(HD))
                    with band("trail"):
                        for h01 in (0, 1):
                            av_mm(aug, pair, e[:, h01 * 512:(h01 + 1) * 512],
                                  kb, h01, 0, 512,
                                  start=(kb == 0), stop=False)
                # mid supertile: blocks nfull, nfull+1 vs upper q-half
                with band("spine"):
                    mid = pq.tile([128, 1024], f32, name="qk")
                    for i in (0, 1):
                        for h01 in (0, 1):
                            qk_mm(mid[:, (h01 * 2 + i) * 256:(h01 * 2 + i + 1) * 256],
                                  pair, nfull + i, q0 + 256, 256, h01)
                    em = epool.tile([128, 1024], bf16, name="e")
                    nc.scalar.activation(em[:], mid[:],
                                         mybir.ActivationFunctionType.Exp,
                                         scale=1.0 / np.sqrt(HD))
                with band("trail"):
                    for i in (0, 1):
                        for h01 in (0, 1):
                            av_mm(aug, pair, em[:, (h01 * 2 + i) * 256:(h01 * 2 + i + 1) * 256],
                                  nfull + i, h01, 256, 256,
                                  start=(nfull == 0 and i == 0), stop=False)
                # two diagonal bands (order 1 then 0; see PSUM group note in
                # the baseline: band0's start must follow band1's close when
                # nfull == 0)
                band_last_av = None
                band0_first_av = None
                for u in (1, 0):
                    with band("spine"):
                        bd = pq.tile([128, 1024], f32, name="qk")
                        for i in (0, 1):
                            for h01 in (0, 1):
                                qk_mm(bd[:, (h01 * 2 + i) * 256:(h01 * 2 + i + 1) * 256],
                                      pair, nfull + 2 * u + i, q0 + u * 256, 256, h01)
                        eb = epool.tile([128, 1024], bf16, name="e")
                        nc.scalar.activation(eb[:], bd[:],
                                             mybir.ActivationFunctionType.Exp,
                                             scale=1.0 / np.sqrt(HD))
                        nc.vector.tensor_mul(eb[:], eb[:], mask_sb[:])
                    with band("trail"):
                        for i in (0, 1):
                            for h01 in (0, 1):
                                av = av_mm(aug, pair,
                                           eb[:, (h01 * 2 + i) * 256:(h01 * 2 + i + 1) * 256],
                                           nfull + 2 * u + i, h01, u * 256, 256,
                                           start=(nfull == 0 and u == 0 and i == 0),
                                           stop=(i == 1))
                                if u == 1:
                                    band_last_av = av
                                elif band0_first_av is None:
                                    band0_first_av = av
                if nfull == 0 and band_last_av is not None:
                    tile.add_dep_helper(band0_first_av.ins, band_last_av.ins,
                                        reason="bank group: band0 start after band1 closes")
                # normalize in half-columns (pipelines recip/bcast/mul) and
                # ship y^T to the gather bounce buffer.  The broadcast is a
                # stride-0 DMA issued from the DVE queue right after the
                # reciprocal it consumes, keeping Pool free for the
                # collective chain.
                with band("trail"):
                    yt = small.tile([64, 1024], bf16, name="yt")
                    for hf in (0, 1):
                        cs = hf * 512
                        recip = small.tile([1, 512], bf16, name="recip")
                        with nc.allow_low_precision(reason="softmax denom in bf16 is within tolerance"):
                            nc.vector.reciprocal(recip[:], aug[64:65, cs:cs + 512])
                        bc = small.tile([64, 512], bf16, name="bc")
                        nc.gpsimd.partition_broadcast(bc[:], recip[:])
                        nc.vector.tensor_mul(yt[:, cs:cs + 512],
                                             aug[0:64, cs:cs + 512], bc[:])
                    if pair == 0:
                        th, tq = divmod(q0, 1024)
                        nc.sync.dma_start(
                            agin0[th].rearrange("(h d) t -> d h t", h=2)[:, :, tq:tq + QT],
                            yt.rearrange("d (h t) -> d h t", h=2))
                    else:
                        nc.sync.dma_start(
                            agin1[qi].rearrange("(h d) t -> d h t", h=2),
                            yt.rearrange("d (h t) -> d h t", h=2))

            # ---- gather + o_proj helpers ----
            # Pool hosts ONLY the collective chain (collectives + the
            # gathered->SBUF DMAs), in dependency order: every wait on that
            # queue blocks only later links of the same chain, which are
            # gated by the serial collective device anyway.
            yg0 = {}
            yg1 = {}

            def cc(agin_t, agout_t):
                with band("trail"):
                    nc.gpsimd.collective_compute(
                        "AllGather", mybir.AluOpType.bypass,
                        replica_groups=GROUPS,
                        ins=[agin_t.opt()], outs=[agout_t.opt()])

            def yg0_dma(th):
                with band("trail"):
                    t = yg0p.tile([128, 4 * 1024], bf16, name="yg0")
                    nc.sync.dma_start(
                        t.rearrange("p (r t) -> p r t", r=4),
                        agout0[th].rearrange("(r p) t -> p r t", r=4))
                    yg0[th] = t

            def yg1_dma(qi):
                with band("trail"):
                    t = yg1p.tile([128, 4 * QT], bf16, name="yg1")
                    nc.sync.dma_start(
                        t.rearrange("p (r t) -> p r t", r=4),
                        agout1[qi].rearrange("(r p) t -> p r t", r=4))
                    yg1[qi] = t

            def oproj_rhs(cb, n):
                r, p = divmod(cb, 2)
                if p == 0:
                    th, tq = divmod(n * QT, 1024)
                    return yg0[th][:, r * 1024 + tq: r * 1024 + tq + QT]
                return yg1[n][:, r * QT:(r + 1) * QT]

            def oproj_tile(n, m):
                """out^T[m*128:(m+1)*128, n*QT:+QT]; even c-blocks (pair 0)
                first so only the odd half waits on gather1(n)."""
                with band("tail"):
                    ps = pp.tile([128, 512], f32, name="pp")
                    cbs = [0, 2, 4, 6, 1, 3, 5, 7]
                    for j, cb in enumerate(cbs):
                        nc.tensor.matmul(
                            ps[:],
                            lhsT=wo_sb[:, cb * SH + m * 128: cb * SH + (m + 1) * 128],
                            rhs=oproj_rhs(cb, n),
                            start=(j == 0), stop=(j == len(cbs) - 1))
                    st = stp.tile([128, 512], bf16, name="st")
                    nc.scalar.activation(st[:], ps[:],
                                         mybir.ActivationFunctionType.Copy)
                    nc.sync.dma_start(
                        out[m * 128:(m + 1) * 128, n * QT: n * QT + QT], st[:])

            # ---- emission schedule ----
            # pair 0: stay one proj tile ahead of the spine; v chunks trail.
            # Collectives and gathered->SBUF DMAs are emitted one/two query
            # tiles after their inputs land so their queue-residency waits
            # are short (an in-order sequencer in a long SemWait stalls
            # every later instruction on that queue).
            qk_proj_tile(0, wq_sb, qt_sb, 0, "spine")
            qk_proj_tile(0, wk_sb, kt_sb, 0, "spine")
            for tch in range(4):
                v_chunk(tch)
            for qi in range(nqt):
                if qi + 1 < nqt:
                    qk_proj_tile(0, wq_sb, qt_sb, qi + 1, "spine")
                    qk_proj_tile(0, wk_sb, kt_sb, qi + 1, "spine")
                    for tch in range(4 * (qi + 1), 4 * (qi + 2)):
                        v_chunk(tch)
                attention_qi(0, qi)
                if qi == 2:
                    cc(agin0[0], agout0[0])
                    # pair-1 projections become available fill work early
                    for n in range(nqt):
                        qk_proj_tile(1, wq_sb, qt_sb, n, "fill")
                        qk_proj_tile(1, wk_sb, kt_sb, n, "fill")
            for qi in range(nqt):
                attention_qi(1, qi)
                if qi == 0:
                    cc(agin0[1], agout0[1])
                    yg0_dma(0)
                elif qi == 1:
                    cc(agin1[0], agout1[0])
                    yg0_dma(1)
                elif qi == 2:
                    cc(agin1[1], agout1[1])
                    yg1_dma(0)
                    oproj_tile(0, 0)
                    oproj_tile(0, 1)
                else:
                    cc(agin1[2], agout1[2])
                    yg1_dma(1)
                    oproj_tile(1, 0)
                    oproj_tile(1, 1)
            cc(agin1[3], agout1[3])
            yg1_dma(2)
            oproj_tile(2, 0)
            oproj_tile(2, 1)
            # keep the PE warm across the tail's collective waits
            warmup(100, pq, [128, 1024], "qk")
            yg1_dma(3)
            oproj_tile(3, 0)
            oproj_tile(3, 1)

    nc.compile()
    return nc


def _masks_np():
    """Diagonal causal mask: [ki, qi] = qi >= ki, duplicated along the free
    axis for the two packed heads."""
    ki = np.arange(128)[:, None]
    qi = np.arange(128)[None, :]
    tri = (qi >= ki).astype(np.float32)
    ones = np.ones((128, 128), np.float32)
    zeros = np.zeros((128, 128), np.float32)
    lo = np.concatenate([tri, ones], axis=1)    # lower k-block of a band
    hi = np.concatenate([zeros, tri], axis=1)   # upper k-block of a band
    return np.concatenate([lo, hi, lo, hi], axis=1).astype(BF16)  # [128, 1024]


def _block(a, w):
    """[C, w] -> [128, NCB*w] partition-blocked bf16."""
    return np.ascontiguousarray(
        a.reshape(NCB, 128, w).transpose(1, 0, 2).reshape(128, NCB * w)).astype(BF16)


def _block_pairs(a):
    """[C, SH] -> [128, 2*NCB*128] partition-blocked, pair-major bf16 so each
    head pair's weights are one contiguous half."""
    b = a.reshape(NCB, 128, SH).transpose(1, 0, 2)          # [128, NCB, SH]
    b = b.reshape(128, NCB, 2, 128).transpose(0, 2, 1, 3)   # [128, pair, NCB, 128]
    return np.ascontiguousarray(b.reshape(128, NCB * SH)).astype(BF16)


def _prep_inputs(x, Wq, Wk, Wv, Wo, t_len):
    masks = _masks_np()
    in_maps = []
    for c in range(N_CORES):
        b, hg = divmod(c, 4)
        sl = slice(hg * SH, (hg + 1) * SH)
        in_maps.append({
            "xT": _block(x[b].T, t_len),
            "wqT": _block_pairs(Wq[sl, :].T),
            "wkT": _block_pairs(Wk[sl, :].T),
            "wvT": _block(Wv[sl, :].T, SH),
            "woT": _block(Wo[sl, :].T, SH),
            "masks": masks,
        })
    return in_maps


def _assemble(results, t_len):
    out = np.empty((B, t_len, C), dtype=np.float32)
    for c in range(N_CORES):
        b, hg = divmod(c, 4)
        out[b, :, hg * SH:(hg + 1) * SH] = results[c]["out"].T.astype(np.float32)
    return out


def get_nc(t_len=T):
    if t_len not in _CACHE:
        _CACHE[t_len] = _build(t_len)
    return _CACHE[t_len]


def kernel(x, Wq, Wk, Wv, Wo):
    from concourse import bass_utils
    x = np.asarray(x, dtype=np.float32)
    nc = get_nc(T)
    in_maps = _prep_inputs(x, np.asarray(Wq), np.asarray(Wk), np.asarray(Wv),
                           np.asarray(Wo), T)
    res = bass_utils.run_bass_kernel_spmd(nc, in_maps, core_ids=list(range(N_CORES)))
    return _assemble(res.results, T)
